# revision 30
# baseline (speedup 1.0000x reference)
"""Trainium2 Bass kernel for BidirectionalAttention.

Math (per batch b):
    xf = x[b].reshape(C, N)                    # C=256, N=4096
    q = Wq @ xf + bq ; k = Wk @ xf + bk        # [32, N]
    v = Wv @ xf + bv                           # [256, N]
    A = softmax_m(q^T k)                       # softmax over keys m
    out = v @ A^T ; y = x + gamma * out        # returned twice

Sharding: 8 cores = (batch b = core//2) x (query-half = core%2).
Attention is permutation-invariant over keys, so the host rotates each
core's image so its query half is always columns 0..2047 — one program
serves all cores.

On-core layout: scores are computed transposed (S^T[m, n]) so exp(S^T)
already has the contraction dim (m) on partitions for the second matmul
U^T[n, c] = sum_m E^T[m, n] * vT[m, c].  vT carries an appended ones
column so the same matmul chain yields the softmax denominator Z[n] for
free.  bv is factored out analytically (U = U_raw + Z*bv, so
U/Z = U_raw/Z + bv) and folded into the residual base
xgb = x + gamma*bv (a per-partition ACT bias in [c, n] layout).
Normalization + gamma are a per-partition scale on U^T; the [n, c] ->
[c, n] transpose runs on the DMA xbar (dma_start_transpose) mid-kernel
(PE saturated, latency hidden) and on the PE for the last chunk
(latency exposed, PE idle); the residual add runs on gpsimd.  exp is
split 50/50 between the scalar engine (exact) and the vector engine
(Schraudolph int16 bit-trick -> bf16) so neither ACT throughput nor
the scores-PSUM recycle ever paces the AV matmul chain, which runs at
the bf16 streaming roofline (~110 ns per 128x128xFD=257 MM).  Score
matmuls (K=32) are packed 4-up into the PE via tile_position row
groups.  x is pre-cast to bf16 on the host (halves the x DMA; the
residual uses bf16 x, rel err ~4e-3 at gamma=0).  A two-group skew
between scores/exp and the AV drain absorbs engine jitter; attention
runs HAM-warm end to end.
"""

import numpy as np

C = 256
C8 = 32
NPIX = 4096     # 64*64
NQ = 2048       # queries per core
B = 4
NCORES = 8
MT = NPIX // 128   # 32 key tiles
NCH = NQ // 512    # 4 query chunks per core
NG = MT // 4       # 8 groups of 4 key tiles

# Schraudolph bf16-exp constants: bits16 = s*2^7*log2(e) + (127*2^7 - adj)
SCHRAU_A = 184.66500854
SCHRAU_C = 16249.0

_cache = {}


def _build():
    import concourse.bacc as bacc
    import concourse.tile as tile
    from concourse import mybir
    from contextlib import ExitStack

    FP32 = mybir.dt.float32
    BF16 = mybir.dt.bfloat16
    I16 = mybir.dt.int16
    AF = mybir.ActivationFunctionType
    OP = mybir.AluOpType

    nc = bacc.Bacc("TRN2", target_bir_lowering=False, debug=False)

    xf_d = nc.dram_tensor("xf", [C, NPIX], BF16, kind="ExternalInput")
    wq_d = nc.dram_tensor("wq", [C8, C], FP32, kind="ExternalInput")
    wk_d = nc.dram_tensor("wk", [C8, C], FP32, kind="ExternalInput")
    wv_d = nc.dram_tensor("wv", [C, C], FP32, kind="ExternalInput")
    bq_d = nc.dram_tensor("bq", [C8, 1], FP32, kind="ExternalInput")
    bk_d = nc.dram_tensor("bk", [C8, 1], FP32, kind="ExternalInput")
    bv_d = nc.dram_tensor("bv", [1, C], FP32, kind="ExternalInput")
    g_d = nc.dram_tensor("gamma", [1, 1], FP32, kind="ExternalInput")
    y_d = nc.dram_tensor("y", [C, NQ], FP32, kind="ExternalOutput")

    with tile.TileContext(nc) as tc, ExitStack() as ctx:
        consts = ctx.enter_context(tc.tile_pool(name="consts", bufs=1))
        stage = ctx.enter_context(tc.tile_pool(name="stage", bufs=2))
        etp = ctx.enter_context(tc.tile_pool(name="et", bufs=6))
        yp = ctx.enter_context(tc.tile_pool(name="yp", bufs=2))
        small = ctx.enter_context(tc.tile_pool(name="small", bufs=8))
        tpt = ctx.enter_context(tc.tile_pool(name="tpt", bufs=4))
        ps_s = ctx.enter_context(tc.tile_pool(name="ps_s", bufs=2, space="PSUM"))
        ps_u = ctx.enter_context(tc.tile_pool(name="ps_u", bufs=4, space="PSUM"))

        # ---- persistent SBUF tensors ----
        ident = consts.tile([128, 128], BF16)
        it = consts.tile([128, 128], mybir.dt.int32)
        nc.gpsimd.iota(it[:], pattern=[[-1, 128]], base=0, channel_multiplier=1)
        nc.vector.tensor_scalar(out=ident[:], in0=it[:], scalar1=0,
                                scalar2=None, op0=OP.is_equal)
        wqt = consts.tile([128, 2, C8], BF16)      # Wq^T, K-chunked
        wkt = consts.tile([128, 2, C8], BF16)
        rv = consts.tile([128, 2, C], BF16)        # Wv^T K-chunks
        bq_sb = consts.tile([128, 1], FP32)   # bq replicated 4x on partitions
        bk_sb = consts.tile([128, 1], FP32)
        gsb = consts.tile([128, 1], FP32)
        gbv = consts.tile([128, 2], FP32)     # gamma*bv as column per c-half
        bvc = consts.tile([128, 2], FP32)     # bv as column per c-half
        # k staggered: partitions 32j..32j+31 hold channels of m-tile 4g+j
        k_sb = consts.tile([128, NG, 128], BF16)
        q_sb = consts.tile([128, NQ], BF16)   # q replicated on 4 row groups
        vt = consts.tile([128, MT, C + 8], BF16)   # v^T tiles + ones col
        xb = consts.tile([128, 2, NPIX], BF16)     # x bf16 (cols 0:NQ = queries)
        xgb = consts.tile([128, 2, NQ], BF16)      # x + gamma*bv (residual base)

        # warm the ACT exp table load under the initial DMA wait
        wrm = small.tile([1, 8], FP32, tag="wrm")
        nc.vector.memset(wrm[:], 0.0)
        wrm2 = small.tile([1, 8], FP32, tag="wrm2")
        nc.scalar.activation(wrm2[:], wrm[:], func=AF.Exp)

        # ones column for the softmax denominator, set once for all m-tiles
        nc.vector.memset(vt[:, :, C:C + 1], 1.0)

        # tiny broadcast DMAs go on the gpsimd SWDGE path so they don't
        # FIFO-block the weight loads on the sync HWDGE ring
        nc.gpsimd.dma_start(out=gsb[:], in_=g_d[:, :].to_broadcast([128, 1]))
        for bd, bt in ((bq_d, bq_sb), (bk_d, bk_sb)):
            for j in range(4):  # replicate bias onto all 4 row groups
                nc.gpsimd.dma_start(out=bt[32 * j:32 * (j + 1), :],
                                    in_=bd[:, :])
        for ch in range(2):  # bv as a per-partition column per c-half
            nc.gpsimd.dma_start(
                out=bvc[:, ch:ch + 1],
                in_=bv_d[0:1, ch * 128:(ch + 1) * 128].rearrange("a b -> b a"))
        nc.vector.tensor_scalar(out=gbv[:], in0=bvc[:], scalar1=gsb[:],
                                scalar2=None, op0=OP.mult)

        # ---- weight prep: cast to bf16, transpose via PE.  x-slice DMAs
        # are interleaved so the big wv transfer doesn't delay slice 0 ----
        SL = 1024

        def issue_x(sl, eng):
            # one 3D DMA per slice, split across the two HWDGE queues
            c0, c1 = sl * SL, (sl + 1) * SL
            eng.dma_start(out=xb[:, :, c0:c1],
                          in_=xf_d[:, c0:c1].rearrange("(ch p) c -> p ch c",
                                                       p=128))

        # slice 0 + odd slices ride the scalar queue (no weight traffic
        # there, and its compute work only starts mid-proj); sync carries
        # the weights first, then the remaining even slices
        for sl in (0, 1, 3):
            issue_x(sl, nc.scalar)

        for wd, wt in ((wq_d, wqt), (wk_d, wkt)):
            wf = stage.tile([C8, C], FP32, tag="wf")
            nc.sync.dma_start(out=wf[:], in_=wd[:, :])
            wfb = stage.tile([C8, C], BF16, tag="wfb")
            nc.vector.tensor_copy(wfb[:], wf[:])
            for kc in range(2):
                tp = ps_u.tile([128, C8], BF16, tag="ut")
                nc.tensor.transpose(tp[:], wfb[:, kc * 128:(kc + 1) * 128],
                                    ident[:C8, :C8])
                nc.any.tensor_copy(wt[:, kc, :], tp[:])

        wvb_tiles = []
        for rh in range(2):
            wvf = stage.tile([128, C], FP32, tag=f"wvf{rh}")
            nc.sync.dma_start(out=wvf[:], in_=wv_d[rh * 128:(rh + 1) * 128, :])
            wvb = stage.tile([128, C], BF16, tag=f"wvb{rh}")
            nc.vector.tensor_copy(wvb[:], wvf[:])
            wvb_tiles.append(wvb)
        for kc in range(2):
            for jh in range(2):
                tp = ps_u.tile([128, 128], BF16, tag="ut")
                nc.tensor.transpose(tp[:],
                                    wvb_tiles[jh][:, kc * 128:(kc + 1) * 128],
                                    ident[:])
                nc.any.tensor_copy(rv[:, kc, jh * 128:(jh + 1) * 128], tp[:])

        issue_x(2, nc.sync)

        # ---- projections, pipelined in 1024-col slices ----
        for sl in range(NPIX // SL):
            c0, c1 = sl * SL, (sl + 1) * SL
            for half in range(SL // 512):
                n0 = c0 + half * 512
                n1 = n0 + 512
                if n1 <= NQ:  # q projection, replicated to all 4 row groups
                    qp = ps_u.tile([128, 512], FP32, tag="ut")
                    for j in range(4):
                        for kc in range(2):
                            nc.tensor.matmul(qp[32 * j:32 * (j + 1), :],
                                             lhsT=wqt[:, kc, :],
                                             rhs=xb[:, kc, n0:n1],
                                             start=(kc == 0), stop=(kc == 1),
                                             tile_position=(0, 32 * j))
                    nc.vector.tensor_scalar_add(q_sb[:, n0:n1], in0=qp[:],
                                                scalar1=bq_sb[:])
                # k projection, staggered: row group j <- m-tile 4g+j
                g = n0 // 512
                kp = ps_u.tile([128, 128], FP32, tag="ut")
                for j in range(4):
                    m0 = n0 + j * 128
                    for kc in range(2):
                        nc.tensor.matmul(kp[32 * j:32 * (j + 1), :],
                                         lhsT=wkt[:, kc, :],
                                         rhs=xb[:, kc, m0:m0 + 128],
                                         start=(kc == 0), stop=(kc == 1),
                                         tile_position=(0, 32 * j))
                nc.vector.tensor_scalar_add(k_sb[:, g, :], in0=kp[:],
                                            scalar1=bk_sb[:])
            for mt in range(c0 // 128, c1 // 128):
                vp = ps_u.tile([128, C], FP32, tag="ut")
                nc.tensor.matmul(vp[:], lhsT=xb[:, 0, mt * 128:(mt + 1) * 128],
                                 rhs=rv[:, 0, :], start=True, stop=False)
                nc.tensor.matmul(vp[:], lhsT=xb[:, 1, mt * 128:(mt + 1) * 128],
                                 rhs=rv[:, 1, :], start=False, stop=True)
                if mt % 2 == 0:
                    nc.scalar.copy(vt[:, mt, 0:C], vp[:])
                else:
                    nc.vector.tensor_copy(vt[:, mt, 0:C], vp[:])

        # residual base x + gamma*bv (per-partition bias = per-channel);
        # not needed until the first epilogue, so it sits after the proj
        # loop where ACT has slack
        for ch in range(2):
            for hv in range(2):
                nc.scalar.activation(
                    xgb[:, ch, hv * 1024:(hv + 1) * 1024],
                    xb[:, ch, hv * 1024:(hv + 1) * 1024],
                    func=AF.Identity, bias=gbv[:, ch:ch + 1])

        # ---- attention: flat (chunk, group) pipeline with two-group
        # skew, so chunk boundaries never idle ACT ----
        def finish_chunk(pnch, uts, last=False):
            # normalize, scale by gamma, transpose, residual, store.  The
            # transpose runs on the DMA xbar mid-kernel (PE is saturated,
            # DMA latency hides under later chunks) but on the PE for the
            # final chunk (PE is idle there and latency is exposed).
            ys = [yp.tile([128, 512], FP32, tag=f"y{ch}", name=f"ys{ch}")
                  for ch in range(2)]
            ots = []
            for nt in range(4):
                up = uts[nt]
                rz = small.tile([128, 1], FP32, tag="rz")
                nc.vector.reciprocal(rz[:], up[:, C:C + 1])
                rzg = small.tile([128, 1], FP32, tag="rzg")
                nc.vector.tensor_scalar_mul(rzg[:], in0=rz[:],
                                            scalar1=gsb[:])
                ot = small.tile([128, C], BF16, tag="ot")
                nc.vector.tensor_scalar_mul(ot[:], in0=up[:, 0:C],
                                            scalar1=rzg[:])
                if last:
                    ots.append(ot)
                    continue
                ott = tpt.tile([128, 2, 128], BF16, tag=f"tt{nt}")
                nc.sync.dma_start_transpose(ott[:], ot[:])
                ots.append(ott)
            for ch in range(2):
                for nt in range(4):
                    x_ap = xgb[:, ch, pnch * 512 + nt * 128:
                               pnch * 512 + (nt + 1) * 128]
                    y_ap = ys[ch][:, nt * 128:(nt + 1) * 128]
                    if last:  # PE transpose into a recycled ut-psum slot
                        tp = ps_u.tile([128, 128], BF16, tag="ut")
                        nc.tensor.transpose(
                            tp[:], ots[nt][:, ch * 128:(ch + 1) * 128],
                            ident[:])
                        nc.vector.tensor_tensor(out=y_ap, in0=tp[:],
                                                in1=x_ap, op=OP.add)
                    else:
                        nc.gpsimd.tensor_tensor(out=y_ap,
                                                in0=ots[nt][:, ch, :],
                                                in1=x_ap, op=OP.add)
                nc.sync.dma_start(
                    out=y_d[ch * 128:(ch + 1) * 128,
                            pnch * 512:(pnch + 1) * 512],
                    in_=ys[ch][:])

        def emit_ut(pnch, pg, pets, uts, last):
            for j in range(4):
                tl, i16 = pets[j // 2]
                for nt in range(4):
                    lh = tl[:, j % 2, nt * 128:(nt + 1) * 128]
                    if i16:
                        lh = lh.bitcast(BF16)
                    nc.tensor.matmul(
                        uts[nt][:], lhsT=lh,
                        rhs=vt[:, 4 * pg + j, 0:C + 1],
                        start=(pg == 0 and j == 0),
                        stop=(pg == NG - 1 and j == 3))
            if pg == NG - 1:
                finish_chunk(pnch, uts, last=last)

        def drain_pend(pend, uts_cur, last=False):
            pnch, pg, pets = pend
            if pg == 0:
                uts_cur = [ps_u.tile([128, C + 1], FP32, tag="ut",
                                     name=f"ut{nt}") for nt in range(4)]
            emit_ut(pnch, pg, pets, uts_cur, last and pg == NG - 1)
            return uts_cur

        pend = []  # two-group skew: exp gets ~2 AV-groups of slack
        uts_cur = None
        for nch in range(NCH):
            for g in range(NG):
                n0 = nch * 512
                ss = []
                ets = []
                for pair in range(2):
                    s2 = ps_s.tile([128, 2, 512], FP32, tag="s",
                                   name=f"s2{pair}")
                    ss.append(s2)
                for j in range(4):
                    nc.tensor.matmul(ss[j // 2][:, j % 2, :],
                                     lhsT=k_sb[32 * j:32 * (j + 1), g, :],
                                     rhs=q_sb[32 * j:32 * (j + 1),
                                              n0:n0 + 512],
                                     start=True, stop=True,
                                     tile_position=(32 * j, 0))
                # exp split 50/50: ACT (exact) and DVE (Schraudolph) run
                # concurrently, halving the exp span so the scores PSUM
                # slot recycles before the PE needs it again
                et0 = etp.tile([128, 2, 512], BF16, tag="et0", name="et0")
                nc.scalar.activation(et0[:], ss[0][:], func=AF.Exp)
                ets.append((et0, False))
                et1 = etp.tile([128, 2, 512], I16, tag="eti1", name="eti1")
                nc.vector.tensor_scalar(
                    out=et1[:], in0=ss[1][:],
                    scalar1=SCHRAU_A, scalar2=SCHRAU_C,
                    op0=OP.mult, op1=OP.add)
                ets.append((et1, True))
                pend.append((nch, g, ets))
                if len(pend) > 2:
                    uts_cur = drain_pend(pend.pop(0), uts_cur)
        while pend:
            uts_cur = drain_pend(pend.pop(0), uts_cur, last=len(pend) == 0)

    nc.finalize()
    return nc


def _in_maps(x, Wq, bq, Wk, bk, Wv, bv, gamma):
    import ml_dtypes
    # pre-cast to bf16 on the host: halves the x DMA and feeds the
    # matmuls directly (residual add also uses bf16 x; error ~4e-3)
    x = np.ascontiguousarray(np.asarray(x).astype(ml_dtypes.bfloat16))
    common = {
        "wq": np.ascontiguousarray(np.asarray(Wq, np.float32)),
        "wk": np.ascontiguousarray(np.asarray(Wk, np.float32)),
        "wv": np.ascontiguousarray(np.asarray(Wv, np.float32)),
        "bq": np.ascontiguousarray(np.asarray(bq, np.float32).reshape(C8, 1)),
        "bk": np.ascontiguousarray(np.asarray(bk, np.float32).reshape(C8, 1)),
        "bv": np.ascontiguousarray(np.asarray(bv, np.float32).reshape(1, C)),
        "gamma": np.ascontiguousarray(
            np.asarray(gamma, np.float32).reshape(1, 1)),
    }
    maps = []
    for core in range(NCORES):
        b, h = divmod(core, 2)
        xf = x[b].reshape(C, NPIX)
        if h == 0:
            xr = xf
        else:  # rotate so this core's queries are columns 0..NQ-1
            xr = np.concatenate([xf[:, NQ:], xf[:, :NQ]], axis=1)
        maps.append({"xf": np.ascontiguousarray(xr), **common})
    return maps


def _run(in_maps, trace=False):
    from concourse.bass_utils import run_bass_kernel_spmd
    if "nc" not in _cache:
        _cache["nc"] = _build()
    return run_bass_kernel_spmd(_cache["nc"], in_maps,
                                core_ids=list(range(NCORES)), trace=trace)


def kernel(x, temb=None, Wq=None, bq=None, Wk=None, bk=None, Wv=None,
           bv=None, gamma=None, **_unused):
    res = _run(_in_maps(x, Wq, bq, Wk, bk, Wv, bv, gamma))
    y = np.empty((B, C, 64, 64), np.float32)
    yf = y.reshape(B, C, NPIX)
    for core in range(NCORES):
        b, h = divmod(core, 2)
        yf[b, :, h * NQ:(h + 1) * NQ] = res.results[core]["y"]
    return (y, y)


# revision 31
# speedup vs baseline: 1.0252x; 1.0252x over previous
"""Trainium2 Bass kernel for BidirectionalAttention.

Math (per batch b):
    xf = x[b].reshape(C, N)                    # C=256, N=4096
    q = Wq @ xf + bq ; k = Wk @ xf + bk        # [32, N]
    v = Wv @ xf + bv                           # [256, N]
    A = softmax_m(q^T k)                       # softmax over keys m
    out = v @ A^T ; y = x + gamma * out        # returned twice

Sharding: 8 cores = (batch b = core//2) x (query-half = core%2).
Attention is permutation-invariant over keys, so the host rotates each
core's image so its query half is always columns 0..2047 — one program
serves all cores.

On-core layout: scores are computed transposed (S^T[m, n]) so exp(S^T)
already has the contraction dim (m) on partitions for the second matmul
U^T[n, c] = sum_m E^T[m, n] * vT[m, c].  vT carries an appended ones
column so the same matmul chain yields the softmax denominator Z[n] for
free.  bv is factored out analytically (U = U_raw + Z*bv, so
U/Z = U_raw/Z + bv) and folded into the residual base
xgb = x + gamma*bv (a per-partition ACT bias in [c, n] layout).
Normalization + gamma are a per-partition scale on U^T; the [n, c] ->
[c, n] transpose runs on the DMA xbar (dma_start_transpose) mid-kernel
(PE saturated, latency hidden) and on the PE for the last chunk
(latency exposed, PE idle); the residual add runs on gpsimd.  exp is
split 50/50 between the scalar engine (exact) and the vector engine
(Schraudolph int16 bit-trick -> bf16) so neither ACT throughput nor
the scores-PSUM recycle ever paces the AV matmul chain, which runs at
the bf16 streaming roofline (~110 ns per 128x128xFD=257 MM).  Score
matmuls (K=32) are packed 4-up into the PE via tile_position row
groups.  x is pre-cast to bf16 on the host (halves the x DMA; the
residual uses bf16 x, rel err ~4e-3 at gamma=0).  A two-group skew
between scores/exp and the AV drain absorbs engine jitter; attention
runs HAM-warm end to end.
"""

import numpy as np

C = 256
C8 = 32
NPIX = 4096     # 64*64
NQ = 2048       # queries per core
B = 4
NCORES = 8
MT = NPIX // 128   # 32 key tiles
NCH = NQ // 512    # 4 query chunks per core
NG = MT // 4       # 8 groups of 4 key tiles

# Schraudolph bf16-exp constants: bits16 = s*2^7*log2(e) + (127*2^7 - adj)
SCHRAU_A = 184.66500854
SCHRAU_C = 16249.0

_cache = {}


def _build():
    import concourse.bacc as bacc
    import concourse.tile as tile
    from concourse import mybir
    from contextlib import ExitStack

    FP32 = mybir.dt.float32
    BF16 = mybir.dt.bfloat16
    I16 = mybir.dt.int16
    AF = mybir.ActivationFunctionType
    OP = mybir.AluOpType

    nc = bacc.Bacc("TRN2", target_bir_lowering=False, debug=False)

    xf_d = nc.dram_tensor("xf", [C, NPIX], BF16, kind="ExternalInput")
    wq_d = nc.dram_tensor("wq", [C8, C], FP32, kind="ExternalInput")
    wk_d = nc.dram_tensor("wk", [C8, C], FP32, kind="ExternalInput")
    wv_d = nc.dram_tensor("wv", [C, C], FP32, kind="ExternalInput")
    bq_d = nc.dram_tensor("bq", [C8, 1], FP32, kind="ExternalInput")
    bk_d = nc.dram_tensor("bk", [C8, 1], FP32, kind="ExternalInput")
    bv_d = nc.dram_tensor("bv", [1, C], FP32, kind="ExternalInput")
    g_d = nc.dram_tensor("gamma", [1, 1], FP32, kind="ExternalInput")
    y_d = nc.dram_tensor("y", [C, NQ], FP32, kind="ExternalOutput")

    with tile.TileContext(nc) as tc, ExitStack() as ctx:
        consts = ctx.enter_context(tc.tile_pool(name="consts", bufs=1))
        stage = ctx.enter_context(tc.tile_pool(name="stage", bufs=2))
        etp = ctx.enter_context(tc.tile_pool(name="et", bufs=6))
        yp = ctx.enter_context(tc.tile_pool(name="yp", bufs=2))
        small = ctx.enter_context(tc.tile_pool(name="small", bufs=8))
        tpt = ctx.enter_context(tc.tile_pool(name="tpt", bufs=4))
        ps_s = ctx.enter_context(tc.tile_pool(name="ps_s", bufs=2, space="PSUM"))
        ps_u = ctx.enter_context(tc.tile_pool(name="ps_u", bufs=4, space="PSUM"))

        # ---- persistent SBUF tensors ----
        ident = consts.tile([128, 128], BF16)
        it = consts.tile([128, 128], mybir.dt.int32)
        nc.gpsimd.iota(it[:], pattern=[[-1, 128]], base=0, channel_multiplier=1)
        nc.vector.tensor_scalar(out=ident[:], in0=it[:], scalar1=0,
                                scalar2=None, op0=OP.is_equal)
        wqt = consts.tile([128, 2, C8], BF16)      # Wq^T, K-chunked
        wkt = consts.tile([128, 2, C8], BF16)
        rv = consts.tile([128, 2, C], BF16)        # Wv^T K-chunks
        bq_sb = consts.tile([128, 1], FP32)   # bq replicated 4x on partitions
        bk_sb = consts.tile([128, 1], FP32)
        gsb = consts.tile([128, 1], FP32)
        gbv = consts.tile([128, 2], FP32)     # gamma*bv as column per c-half
        bvc = consts.tile([128, 2], FP32)     # bv as column per c-half
        # k staggered: partitions 32j..32j+31 hold channels of m-tile 4g+j
        k_sb = consts.tile([128, NG, 128], BF16)
        q_sb = consts.tile([128, NQ], BF16)   # q replicated on 4 row groups
        vt = consts.tile([128, MT, C + 8], BF16)   # v^T tiles + ones col
        xb = consts.tile([128, 2, NPIX], BF16)     # x bf16 (cols 0:NQ = queries)
        xgb = consts.tile([128, 2, NQ], BF16)      # x + gamma*bv (residual base)

        # warm the ACT exp table load under the initial DMA wait
        wrm = small.tile([1, 8], FP32, tag="wrm")
        nc.vector.memset(wrm[:], 0.0)
        wrm2 = small.tile([1, 8], FP32, tag="wrm2")
        nc.scalar.activation(wrm2[:], wrm[:], func=AF.Exp)

        # ones column for the softmax denominator, set once for all m-tiles
        nc.vector.memset(vt[:, :, C:C + 1], 1.0)

        # tiny broadcast DMAs go on the gpsimd SWDGE path so they don't
        # FIFO-block the weight loads on the sync HWDGE ring
        nc.gpsimd.dma_start(out=gsb[:], in_=g_d[:, :].to_broadcast([128, 1]))
        for bd, bt in ((bq_d, bq_sb), (bk_d, bk_sb)):
            for j in range(4):  # replicate bias onto all 4 row groups
                nc.gpsimd.dma_start(out=bt[32 * j:32 * (j + 1), :],
                                    in_=bd[:, :])
        for ch in range(2):  # bv as a per-partition column per c-half
            nc.gpsimd.dma_start(
                out=bvc[:, ch:ch + 1],
                in_=bv_d[0:1, ch * 128:(ch + 1) * 128].rearrange("a b -> b a"))
        nc.vector.tensor_scalar(out=gbv[:], in0=bvc[:], scalar1=gsb[:],
                                scalar2=None, op0=OP.mult)

        # ---- weight prep: cast to bf16, transpose via PE.  x-slice DMAs
        # are interleaved so the big wv transfer doesn't delay slice 0 ----
        SL = 1024

        def issue_x(sl, eng):
            # one 3D DMA per slice, split across the two HWDGE queues
            c0, c1 = sl * SL, (sl + 1) * SL
            eng.dma_start(out=xb[:, :, c0:c1],
                          in_=xf_d[:, c0:c1].rearrange("(ch p) c -> p ch c",
                                                       p=128))

        # slice 0 + odd slices ride the scalar queue (no weight traffic
        # there, and its compute work only starts mid-proj); sync carries
        # the weights first, then the remaining even slices
        for sl in (0, 1, 3):
            issue_x(sl, nc.scalar)

        # dummy matmuls fill the initial DMA wait so the PE's HAM clock
        # gate reaches 8/8 before the projections start (~12 cold MMs
        # cover the ~2.7us gap; they retire before the first weights land)
        wrmp = ps_u.tile([128, 128], FP32, tag="ut")
        for _i in range(12):
            nc.tensor.matmul(wrmp[:], lhsT=ident[:], rhs=ident[:],
                             start=True, stop=True)

        for wd, wt in ((wq_d, wqt), (wk_d, wkt)):
            wf = stage.tile([C8, C], FP32, tag="wf")
            nc.sync.dma_start(out=wf[:], in_=wd[:, :])
            wfb = stage.tile([C8, C], BF16, tag="wfb")
            nc.vector.tensor_copy(wfb[:], wf[:])
            for kc in range(2):
                tp = ps_u.tile([128, C8], BF16, tag="ut")
                nc.tensor.transpose(tp[:], wfb[:, kc * 128:(kc + 1) * 128],
                                    ident[:C8, :C8])
                nc.any.tensor_copy(wt[:, kc, :], tp[:])

        wvb_tiles = []
        for rh in range(2):
            wvf = stage.tile([128, C], FP32, tag=f"wvf{rh}")
            nc.sync.dma_start(out=wvf[:], in_=wv_d[rh * 128:(rh + 1) * 128, :])
            wvb = stage.tile([128, C], BF16, tag=f"wvb{rh}")
            nc.vector.tensor_copy(wvb[:], wvf[:])
            wvb_tiles.append(wvb)
        for kc in range(2):
            for jh in range(2):
                tp = ps_u.tile([128, 128], BF16, tag="ut")
                nc.tensor.transpose(tp[:],
                                    wvb_tiles[jh][:, kc * 128:(kc + 1) * 128],
                                    ident[:])
                nc.any.tensor_copy(rv[:, kc, jh * 128:(jh + 1) * 128], tp[:])

        issue_x(2, nc.sync)

        # ---- projections, pipelined in 1024-col slices ----
        for sl in range(NPIX // SL):
            c0, c1 = sl * SL, (sl + 1) * SL
            for half in range(SL // 512):
                n0 = c0 + half * 512
                n1 = n0 + 512
                if n1 <= NQ:  # q projection, replicated to all 4 row groups
                    qp = ps_u.tile([128, 512], FP32, tag="ut")
                    for j in range(4):
                        for kc in range(2):
                            nc.tensor.matmul(qp[32 * j:32 * (j + 1), :],
                                             lhsT=wqt[:, kc, :],
                                             rhs=xb[:, kc, n0:n1],
                                             start=(kc == 0), stop=(kc == 1),
                                             tile_position=(0, 32 * j))
                    nc.vector.tensor_scalar_add(q_sb[:, n0:n1], in0=qp[:],
                                                scalar1=bq_sb[:])
                # k projection, staggered: row group j <- m-tile 4g+j
                g = n0 // 512
                kp = ps_u.tile([128, 128], FP32, tag="ut")
                for j in range(4):
                    m0 = n0 + j * 128
                    for kc in range(2):
                        nc.tensor.matmul(kp[32 * j:32 * (j + 1), :],
                                         lhsT=wkt[:, kc, :],
                                         rhs=xb[:, kc, m0:m0 + 128],
                                         start=(kc == 0), stop=(kc == 1),
                                         tile_position=(0, 32 * j))
                nc.vector.tensor_scalar_add(k_sb[:, g, :], in0=kp[:],
                                            scalar1=bk_sb[:])
            for mt in range(c0 // 128, c1 // 128):
                vp = ps_u.tile([128, C], FP32, tag="ut")
                nc.tensor.matmul(vp[:], lhsT=xb[:, 0, mt * 128:(mt + 1) * 128],
                                 rhs=rv[:, 0, :], start=True, stop=False)
                nc.tensor.matmul(vp[:], lhsT=xb[:, 1, mt * 128:(mt + 1) * 128],
                                 rhs=rv[:, 1, :], start=False, stop=True)
                if mt % 2 == 0:
                    nc.scalar.copy(vt[:, mt, 0:C], vp[:])
                else:
                    nc.vector.tensor_copy(vt[:, mt, 0:C], vp[:])

        # residual base x + gamma*bv (per-partition bias = per-channel);
        # not needed until the first epilogue, so it sits after the proj
        # loop where ACT has slack
        for ch in range(2):
            for hv in range(2):
                nc.scalar.activation(
                    xgb[:, ch, hv * 1024:(hv + 1) * 1024],
                    xb[:, ch, hv * 1024:(hv + 1) * 1024],
                    func=AF.Identity, bias=gbv[:, ch:ch + 1])

        # ---- attention: flat (chunk, group) pipeline with two-group
        # skew, so chunk boundaries never idle ACT ----
        def finish_chunk(pnch, uts, last=False):
            # normalize, scale by gamma, transpose, residual, store.  The
            # transpose runs on the DMA xbar mid-kernel (PE is saturated,
            # DMA latency hides under later chunks) but on the PE for the
            # final chunk (PE is idle there and latency is exposed).
            ys = [yp.tile([128, 512], FP32, tag=f"y{ch}", name=f"ys{ch}")
                  for ch in range(2)]
            ots = []
            for nt in range(4):
                up = uts[nt]
                rz = small.tile([128, 1], FP32, tag="rz")
                nc.vector.reciprocal(rz[:], up[:, C:C + 1])
                rzg = small.tile([128, 1], FP32, tag="rzg")
                nc.vector.tensor_scalar_mul(rzg[:], in0=rz[:],
                                            scalar1=gsb[:])
                ot = small.tile([128, C], BF16, tag="ot")
                nc.vector.tensor_scalar_mul(ot[:], in0=up[:, 0:C],
                                            scalar1=rzg[:])
                if last:
                    ots.append(ot)
                    continue
                ott = tpt.tile([128, 2, 128], BF16, tag=f"tt{nt}")
                nc.sync.dma_start_transpose(ott[:], ot[:])
                ots.append(ott)
            for ch in range(2):
                for nt in range(4):
                    x_ap = xgb[:, ch, pnch * 512 + nt * 128:
                               pnch * 512 + (nt + 1) * 128]
                    y_ap = ys[ch][:, nt * 128:(nt + 1) * 128]
                    if last:  # PE transpose into a recycled ut-psum slot
                        tp = ps_u.tile([128, 128], BF16, tag="ut")
                        nc.tensor.transpose(
                            tp[:], ots[nt][:, ch * 128:(ch + 1) * 128],
                            ident[:])
                        nc.vector.tensor_tensor(out=y_ap, in0=tp[:],
                                                in1=x_ap, op=OP.add)
                    else:
                        nc.gpsimd.tensor_tensor(out=y_ap,
                                                in0=ots[nt][:, ch, :],
                                                in1=x_ap, op=OP.add)
                nc.sync.dma_start(
                    out=y_d[ch * 128:(ch + 1) * 128,
                            pnch * 512:(pnch + 1) * 512],
                    in_=ys[ch][:])

        def emit_ut(pnch, pg, pets, uts, last):
            for j in range(4):
                tl, i16 = pets[j // 2]
                for nt in range(4):
                    lh = tl[:, j % 2, nt * 128:(nt + 1) * 128]
                    if i16:
                        lh = lh.bitcast(BF16)
                    nc.tensor.matmul(
                        uts[nt][:], lhsT=lh,
                        rhs=vt[:, 4 * pg + j, 0:C + 1],
                        start=(pg == 0 and j == 0),
                        stop=(pg == NG - 1 and j == 3))
            if pg == NG - 1:
                finish_chunk(pnch, uts, last=last)

        def drain_pend(pend, uts_cur, last=False):
            pnch, pg, pets = pend
            if pg == 0:
                uts_cur = [ps_u.tile([128, C + 1], FP32, tag="ut",
                                     name=f"ut{nt}") for nt in range(4)]
            emit_ut(pnch, pg, pets, uts_cur, last and pg == NG - 1)
            return uts_cur

        pend = []  # two-group skew: exp gets ~2 AV-groups of slack
        uts_cur = None
        for nch in range(NCH):
            for g in range(NG):
                n0 = nch * 512
                ss = []
                ets = []
                for pair in range(2):
                    s2 = ps_s.tile([128, 2, 512], FP32, tag="s",
                                   name=f"s2{pair}")
                    ss.append(s2)
                for j in range(4):
                    nc.tensor.matmul(ss[j // 2][:, j % 2, :],
                                     lhsT=k_sb[32 * j:32 * (j + 1), g, :],
                                     rhs=q_sb[32 * j:32 * (j + 1),
                                              n0:n0 + 512],
                                     start=True, stop=True,
                                     tile_position=(32 * j, 0))
                # exp split 50/50: ACT (exact) and DVE (Schraudolph) run
                # concurrently, halving the exp span so the scores PSUM
                # slot recycles before the PE needs it again
                et0 = etp.tile([128, 2, 512], BF16, tag="et0", name="et0")
                nc.scalar.activation(et0[:], ss[0][:], func=AF.Exp)
                ets.append((et0, False))
                et1 = etp.tile([128, 2, 512], I16, tag="eti1", name="eti1")
                nc.vector.tensor_scalar(
                    out=et1[:], in0=ss[1][:],
                    scalar1=SCHRAU_A, scalar2=SCHRAU_C,
                    op0=OP.mult, op1=OP.add)
                ets.append((et1, True))
                pend.append((nch, g, ets))
                if len(pend) > 2:
                    uts_cur = drain_pend(pend.pop(0), uts_cur)
        while pend:
            uts_cur = drain_pend(pend.pop(0), uts_cur, last=len(pend) == 0)

    nc.finalize()
    return nc


def _in_maps(x, Wq, bq, Wk, bk, Wv, bv, gamma):
    import ml_dtypes
    # pre-cast to bf16 on the host: halves the x DMA and feeds the
    # matmuls directly (residual add also uses bf16 x; error ~4e-3)
    x = np.ascontiguousarray(np.asarray(x).astype(ml_dtypes.bfloat16))
    common = {
        "wq": np.ascontiguousarray(np.asarray(Wq, np.float32)),
        "wk": np.ascontiguousarray(np.asarray(Wk, np.float32)),
        "wv": np.ascontiguousarray(np.asarray(Wv, np.float32)),
        "bq": np.ascontiguousarray(np.asarray(bq, np.float32).reshape(C8, 1)),
        "bk": np.ascontiguousarray(np.asarray(bk, np.float32).reshape(C8, 1)),
        "bv": np.ascontiguousarray(np.asarray(bv, np.float32).reshape(1, C)),
        "gamma": np.ascontiguousarray(
            np.asarray(gamma, np.float32).reshape(1, 1)),
    }
    maps = []
    for core in range(NCORES):
        b, h = divmod(core, 2)
        xf = x[b].reshape(C, NPIX)
        if h == 0:
            xr = xf
        else:  # rotate so this core's queries are columns 0..NQ-1
            xr = np.concatenate([xf[:, NQ:], xf[:, :NQ]], axis=1)
        maps.append({"xf": np.ascontiguousarray(xr), **common})
    return maps


def _run(in_maps, trace=False):
    from concourse.bass_utils import run_bass_kernel_spmd
    if "nc" not in _cache:
        _cache["nc"] = _build()
    return run_bass_kernel_spmd(_cache["nc"], in_maps,
                                core_ids=list(range(NCORES)), trace=trace)


def kernel(x, temb=None, Wq=None, bq=None, Wk=None, bk=None, Wv=None,
           bv=None, gamma=None, **_unused):
    res = _run(_in_maps(x, Wq, bq, Wk, bk, Wv, bv, gamma))
    y = np.empty((B, C, 64, 64), np.float32)
    yf = y.reshape(B, C, NPIX)
    for core in range(NCORES):
        b, h = divmod(core, 2)
        yf[b, :, h * NQ:(h + 1) * NQ] = res.results[core]["y"]
    return (y, y)
